# revision 14
# baseline (speedup 1.0000x reference)
"""Tied-attention (MSA-style) kernel for 8 TRN2 NeuronCores — v2 (fp8 DR).

Problem: x (32,1024,256) f32; q/kv projections; tied attention over the
r=32 MSA-row dim (logits summed over r); softmax; out-projection + bias.

Sharding: tensor-parallel by heads (8 heads -> 1 head per core). Each core
computes q/k/v for its head from the full (host-pre-transposed, bf16-cast)
x, accumulates its head's tied logits S^T = sum_r k_r q_r^T locally (no
collective), softmaxes along the PSUM partition axis via a ones-matmul,
applies attention, then four AllToAlls (one per 8-row group) redistribute
per-head outputs into per-core row shards, overlapping the attention
compute; each core finishes with the full output projection for its rows.

v2 over v1: q/k/v/P are quantized to fp8 e4m3 (x16 scale on q/k/v) and the
two big attention matmuls (dots, attn@v) run in DoubleRow fp8 — contraction
256 per pass (two r-pairs / two j-blocks as the [K,2,free] dim-1) for ~2x
PE throughput. Projections and out-projection stay bf16; the A2A payload
stays bf16 (fp8 transport costs too much accuracy post-averaging).

v is produced head-transposed and flipped to row-major with PE-mode
transposes (DMA transpose serializes the whole DMA subsystem, measured
12us/pair stalls in v1 - avoided).
"""
import numpy as np
import ml_dtypes

import concourse.bacc as bacc
import concourse.mybir as mybir
import concourse.tile as tile
from concourse.bass_utils import run_bass_kernel_spmd

dt = mybir.dt
BF16 = ml_dtypes.bfloat16

H, D, R, N, DIM = 8, 64, 32, 1024, 256
INNER = H * D          # 512
ROWS = R * N           # 32768
NPAIR = R // 2         # 16
NCORES = 8
RL = R // NCORES       # 4 rows of r per core after AllToAll
SCALE = (D ** -0.5) * (R ** -0.5)
QS = 16.0              # q/k/v fp8 pre-scale

_NC_CACHE = None


def _build(loop_n=None):
    nc = bacc.Bacc("TRN2", target_bir_lowering=False, debug=False, num_devices=NCORES)

    xt = nc.dram_tensor("xt", [DIM, ROWS], dt.bfloat16, kind="ExternalInput")
    wq = nc.dram_tensor("wq", [DIM, D], dt.bfloat16, kind="ExternalInput")
    wk = nc.dram_tensor("wk", [DIM, D], dt.bfloat16, kind="ExternalInput")
    wv = nc.dram_tensor("wv", [DIM, D], dt.bfloat16, kind="ExternalInput")
    wout = nc.dram_tensor("wout", [INNER, DIM], dt.bfloat16, kind="ExternalInput")
    bias = nc.dram_tensor("bias", [128, 2], dt.float32, kind="ExternalInput")
    ident = nc.dram_tensor("ident", [128, 128], dt.bfloat16, kind="ExternalInput")
    yt = nc.dram_tensor("yt", [DIM, RL * N], dt.float32, kind="ExternalOutput")

    tens = (xt, wq, wk, wv, wout, bias, ident, yt)
    if loop_n is None:
        _emit_body(nc, tens)
    else:
        from concourse.benchmark.neff_loop import build_neff_loop
        build_neff_loop(nc, lambda notif_base: _emit_body(nc, tens),
                        n_iters=loop_n, n_warmup=0)
    nc.finalize()
    return nc


def _emit_body(nc, tens):
    xt, wq, wk, wv, wout, bias, ident, yt = tens
    with tile.TileContext(nc) as tc:
        with (
            tc.tile_pool(name="dram", bufs=1, space="DRAM") as dram,
            tc.tile_pool(name="persist", bufs=1) as per,
            tc.tile_pool(name="xc", bufs=4) as xcp,
            tc.tile_pool(name="stage", bufs=4) as stg,
            tc.tile_pool(name="gio", bufs=2) as gio,
        ):
            # A2A: 3 chunks over row-groups rg=r//8: chunk 0 = rg 0 (rows
            # [0,8)), chunk 1 = rg 1, chunk 2 = rgs 2,3. Chunk c fires when
            # its rows' pairs are done; dest d gets row 8*rg+d in slot rg-2
            # for chunk 2.
            a2a_ins = [dram.tile([NCORES, 1, D, N], dt.bfloat16, name="a2ai0"),
                       dram.tile([NCORES, 1, D, N], dt.bfloat16, name="a2ai1"),
                       dram.tile([NCORES, 2, D, N], dt.bfloat16, name="a2ai2")]
            a2a_outs = [dram.tile([NCORES, 1, D, N], dt.bfloat16, name="a2ao0"),
                        dram.tile([NCORES, 1, D, N], dt.bfloat16, name="a2ao1"),
                        dram.tile([NCORES, 2, D, N], dt.bfloat16, name="a2ao2")]

            # persistent SBUF tensors
            wq_sb = per.tile([128, 2, D], dt.bfloat16, tag="wq")
            wk_sb = per.tile([128, 2, D], dt.bfloat16, tag="wk")
            wv_sb = per.tile([128, 2, D], dt.bfloat16, tag="wv")
            wout_sb = per.tile([128, 4, DIM], dt.bfloat16, tag="wout")
            bias_sb = per.tile([128, 2], dt.float32, tag="bias")
            ident_sb = per.tile([128, 128], dt.bfloat16, tag="ident")
            # [128, 2, 16] so the DR k-pair stride is 16B (ISA: step%16==0);
            # only [:, :, 0:1] is used as the ones stationary.
            ones2 = per.tile([128, 2, 16], dt.float8e4, tag="ones2")
            den_sb = per.tile([1, N], dt.float32, tag="den")
            bcf_sb = per.tile([128, N], dt.float32, tag="bcf")
            # fp8 DR operands: dim1 is the contraction-pair slot.
            # qts2/kts2[p2][:, s, :] = q/k of r-pair (2*p2+s), layout
            # [(parity,d) 128, s 2, n N].
            qts2 = [per.tile([128, 2, N], dt.float8e4, tag=f"qt{p2}", name=f"qt{p2}")
                    for p2 in range(NPAIR // 2)]
            kts2 = [per.tile([128, 2, N], dt.float8e4, tag=f"kt{p2}", name=f"kt{p2}")
                    for p2 in range(NPAIR // 2)]
            # vs2[p][:, s, c2, :] = v row-major for token block 2*c2+s:
            # [token-in-block 128, s 2, c2 4, (parity,d) 128]
            vs2 = [per.tile([128, 2, 4, 128], dt.float8e4, tag=f"v{p}", name=f"v{p}")
                   for p in range(NPAIR)]
            # pts2[c2][:, s, :] = P^T for j-block 2*c2+s: [j-in-block, s, i]
            pts2 = [per.tile([128, 2, N], dt.float8e4, tag=f"pt{c2}", name=f"pt{c2}")
                    for c2 in range(4)]

            nc.gpsimd.dma_start(wq_sb[:], wq.ap().rearrange("(a p) m -> p a m", p=128))
            nc.gpsimd.dma_start(wk_sb[:], wk.ap().rearrange("(a p) m -> p a m", p=128))
            nc.gpsimd.dma_start(wv_sb[:], wv.ap().rearrange("(a p) m -> p a m", p=128))
            nc.gpsimd.dma_start(wout_sb[:], wout.ap().rearrange("(a p) m -> p a m", p=128))
            nc.gpsimd.dma_start(bias_sb[:], bias[:])
            nc.gpsimd.dma_start(ident_sb[:], ident[:])
            nc.vector.memset(ones2[:], 1.0)

            # warm-up collective: absorbs cross-core start skew and ncfw cold
            # init while phase 1 computes; CC engine only.
            warm_in = dram.tile([1, 64], dt.float32, name="warm_in")
            warm_out = dram.tile([NCORES, 64], dt.float32, name="warm_out")
            nc.gpsimd.collective_compute(
                "AllGather",
                mybir.AluOpType.bypass,
                replica_groups=[list(range(NCORES))],
                ins=[warm_in.opt()],
                outs=[warm_out.opt()],
            )

            # ---- Phase 1: projections q^T,k^T (parity layout, fp8 x16) +
            # v (row major fp8 x16 via PE transposes) ----
            with (
                tc.tile_pool(name="ps_proj", bufs=3, space="PSUM") as psp,
                tc.tile_pool(name="ps_vtr", bufs=2, space="PSUM") as psv,
            ):
                for p in range(NPAIR):
                    xc = [xcp.tile([128, 2 * N], dt.bfloat16, tag="xc", name=f"xc{p}_{i}")
                          for i in range(2)]
                    for kt in range(2):
                        for hf in range(2):
                            nc.sync.dma_start(
                                xc[kt][:, hf * N:(hf + 1) * N],
                                xt[kt * 128:(kt + 1) * 128,
                                   (2 * p + hf) * N:(2 * p + hf + 1) * N])
                    pq = psp.tile([128, N], dt.float32, tag="proj", name=f"pq{p}")
                    pk = psp.tile([128, N], dt.float32, tag="proj", name=f"pk{p}")
                    pv = psp.tile([128, N], dt.float32, tag="proj", name=f"pv{p}")
                    # col-inner issue order: consecutive matmuls target
                    # different PE column groups, so the two 64-wide
                    # stationaries run concurrently in the array.
                    for w_sb, ps in ((wq_sb, pq), (wk_sb, pk), (wv_sb, pv)):
                        for kt in range(2):
                            for nh in range(2):
                                for col, base in ((0, 0), (64, N)):
                                    sl = slice(base + nh * 512, base + nh * 512 + 512)
                                    # partition-disjoint col-tile groups in one
                                    # bank: safe on HW (per-element has_written)
                                    nc.tensor.matmul(
                                        ps[col:col + 64, nh * 512:nh * 512 + 512],
                                        w_sb[:, kt, :], xc[kt][:, sl],
                                        start=(kt == 0), stop=(kt == 1),
                                        tile_position=(0, col),
                                        skip_group_check=True)
                    p2, s = p // 2, p % 2
                    nc.scalar.activation(qts2[p2][:, s, :], pq[:],
                                         mybir.ActivationFunctionType.Identity,
                                         scale=QS)
                    nc.vector.tensor_scalar_mul(kts2[p2][:, s, :], pk[:], QS)
                    vstage = stg.tile([128, N], dt.bfloat16, tag="vstage",
                                      name=f"vst{p}")
                    nc.vector.tensor_copy(vstage[:], pv[:])
                    # PE-transpose v^T (parity,d)xn -> n x (parity,d) in bf16
                    # (fp8 transpose needs output element step 2); quantize to
                    # e4m3 x16 on the PSUM->SBUF evacuation instead.
                    for jc in range(8):
                        pt_ps = psv.tile([128, 128], dt.bfloat16, tag="vtr",
                                         name=f"vtr{p}_{jc}")
                        nc.tensor.transpose(pt_ps[:],
                                            vstage[:, jc * 128:(jc + 1) * 128],
                                            ident_sb[:])
                        dst = vs2[p][:, jc % 2, jc // 2, :]
                        if jc % 2 == 0:
                            nc.vector.tensor_scalar_mul(dst, pt_ps[:], QS)
                        else:
                            nc.scalar.activation(dst, pt_ps[:],
                                                 mybir.ActivationFunctionType.Identity,
                                                 scale=QS)

            # ---- Phase 2: S^T = sum_r k_r q_r^T (DR fp8, 2 r-pairs/pass),
            # softmax -> P fp8 ----
            with (
                tc.tile_pool(name="ps_s", bufs=3, space="PSUM") as pss,
                tc.tile_pool(name="ps_den", bufs=1, space="PSUM") as psd,
            ):
                pden = psd.tile([1, N], dt.float32, tag="den")
                for jc in range(8):
                    ps = pss.tile([128, N], dt.float32, tag="s", name=f"s{jc}")
                    for p2 in range(NPAIR // 2):
                        for ih in range(2):
                            nc.tensor.matmul(
                                ps[:, ih * 512:ih * 512 + 512],
                                kts2[p2][:, :, jc * 128:(jc + 1) * 128],
                                qts2[p2][:, :, ih * 512:ih * 512 + 512],
                                start=(p2 == 0), stop=(p2 == NPAIR // 2 - 1),
                                perf_mode=mybir.MatmulPerfMode.DoubleRow)
                    # P = exp(SCALE/QS^2 * S^T) in e4m3 (values ~[0.3, 2])
                    nc.scalar.activation(pts2[jc // 2][:, jc % 2, :], ps[:],
                                         mybir.ActivationFunctionType.Exp,
                                         scale=SCALE / (QS * QS))
                # den = sum_j P (DR ones-matmul sums both j-blocks per pass)
                for c2 in range(4):
                    for ih in range(2):
                        nc.tensor.matmul(
                            pden[:, ih * 512:ih * 512 + 512],
                            ones2[:, :, 0:1],
                            pts2[c2][:, :, ih * 512:ih * 512 + 512],
                            start=(c2 == 0), stop=(c2 == 3),
                            perf_mode=mybir.MatmulPerfMode.DoubleRow)
                nc.scalar.activation(den_sb[:], pden[:],
                                     mybir.ActivationFunctionType.Identity,
                                     scale=QS)
            # broadcast first, then full-width reciprocal (fast on 128 lanes)
            # bcf = 1/(QS*den) so po*bcf = out exactly (po carries QS from v)
            nc.gpsimd.partition_broadcast(bcf_sb[:], den_sb[:])
            nc.vector.reciprocal(bcf_sb[:], bcf_sb[:])

            # ---- Phase 3 + 4: attention-weighted values (DR fp8, 2 j-blocks
            # per pass); A2A chunk c fires after pairs 4c..4c+3 ----
            with tc.tile_pool(name="ps_av", bufs=3, space="PSUM") as psa:
                for chunk in range(3):
                    prange = (range(4 * chunk, 4 * chunk + 4) if chunk < 2
                              else range(8, 16))
                    for p in prange:
                        po = psa.tile([128, N], dt.float32, tag="av", name=f"av{p}")
                        for c2 in range(4):
                            for ih in range(2):
                                nc.tensor.matmul(
                                    po[:, ih * 512:ih * 512 + 512],
                                    vs2[p][:, :, c2, :],
                                    pts2[c2][:, :, ih * 512:ih * 512 + 512],
                                    start=(c2 == 0), stop=(c2 == 3),
                                    perf_mode=mybir.MatmulPerfMode.DoubleRow)
                        osb = stg.tile([128, N], dt.bfloat16, tag="osb",
                                       name=f"osb{p}")
                        # normalize by the softmax denominator on evacuation
                        nc.vector.tensor_mul(osb[:], po[:], bcf_sb[:])
                        for par in range(2):
                            r = 2 * p + par
                            slot = r // 8 - (chunk if chunk < 2 else 2)
                            nc.sync.dma_start(
                                a2a_ins[chunk][r % 8, slot, :, :],
                                osb[64 * par:64 * par + 64, :])
                    nc.gpsimd.collective_compute(
                        "AllToAll",
                        mybir.AluOpType.bypass,
                        replica_groups=[list(range(NCORES))],
                        ins=[a2a_ins[chunk].opt()],
                        outs=[a2a_outs[chunk].opt()],
                    )

            # ---- Phase 5: y^T = Wout^T out + bias for own 4 r-rows ----
            with tc.tile_pool(name="ps_y", bufs=4, space="PSUM") as psy:
                for rl in range(RL):
                    g = gio.tile([128, 4, N], dt.bfloat16, tag="g", name=f"g{rl}")
                    chunk, sub = (rl, 0) if rl < 2 else (2, rl - 2)
                    for kt in range(4):
                        nc.scalar.dma_start(g[0:64, kt, :],
                                            a2a_outs[chunk][2 * kt, sub, :, :])
                        nc.scalar.dma_start(g[64:128, kt, :],
                                            a2a_outs[chunk][2 * kt + 1, sub, :, :])
                    for m in range(2):
                        sl_m = slice(m * 128, m * 128 + 128)
                        py = psy.tile([128, N], dt.float32, tag="y",
                                      name=f"py{rl}_{m}")
                        for kt in range(4):
                            for nh in range(2):
                                nc.tensor.matmul(py[:, nh * 512:nh * 512 + 512],
                                                 wout_sb[:, kt, sl_m],
                                                 g[:, kt, nh * 512:nh * 512 + 512],
                                                 start=(kt == 0), stop=(kt == 3))
                        ysb = gio.tile([128, N], dt.float32, tag="ysb",
                                       name=f"ysb{rl}_{m}")
                        if m == 0:
                            nc.vector.tensor_scalar_add(ysb[:], py[:],
                                                        bias_sb[:, m:m + 1])
                        else:
                            nc.scalar.activation(ysb[:], py[:],
                                                 mybir.ActivationFunctionType.Identity,
                                                 bias=bias_sb[:, m:m + 1])
                        nc.gpsimd.dma_start(yt[sl_m, rl * N:(rl + 1) * N], ysb[:])


def _make_in_map(x, Wq, Wkv, Wout, bout, core):
    xtb = np.ascontiguousarray(
        np.asarray(x, np.float32).reshape(ROWS, DIM).T).astype(BF16)
    sl = slice(core * D, (core + 1) * D)
    return {
        "xt": xtb,
        "wq": np.ascontiguousarray(np.asarray(Wq, np.float32)[:, sl]).astype(BF16),
        "wk": np.ascontiguousarray(np.asarray(Wkv, np.float32)[:, sl]).astype(BF16),
        "wv": np.ascontiguousarray(
            np.asarray(Wkv, np.float32)[:, INNER + core * D:INNER + (core + 1) * D]
        ).astype(BF16),
        "wout": np.asarray(Wout, np.float32).astype(BF16),
        "bias": np.ascontiguousarray(
            np.asarray(bout, np.float32).reshape(2, 128).T).astype(np.float32),
        "ident": np.eye(128, dtype=BF16),
    }


def kernel(x, Wq, Wkv, Wout, bout, tie_attn_dim):
    global _NC_CACHE
    assert int(tie_attn_dim) == R
    x = np.asarray(x, dtype=np.float32)
    xtb = np.ascontiguousarray(x.reshape(ROWS, DIM).T).astype(BF16)
    Wq = np.asarray(Wq, np.float32)
    Wkv = np.asarray(Wkv, np.float32)
    wout_b = np.asarray(Wout, np.float32).astype(BF16)
    bias_b = np.ascontiguousarray(
        np.asarray(bout, np.float32).reshape(2, 128).T).astype(np.float32)
    ident = np.eye(128, dtype=ml_dtypes.float8_e4m3)

    in_maps = []
    for c in range(NCORES):
        sl = slice(c * D, (c + 1) * D)
        in_maps.append({
            "xt": xtb,
            "wq": np.ascontiguousarray(Wq[:, sl]).astype(BF16),
            "wk": np.ascontiguousarray(Wkv[:, sl]).astype(BF16),
            "wv": np.ascontiguousarray(
                Wkv[:, INNER + c * D:INNER + (c + 1) * D]).astype(BF16),
            "wout": wout_b,
            "bias": bias_b,
            "ident": ident,
        })

    if _NC_CACHE is None:
        _NC_CACHE = _build()
    last_err = None
    for _attempt in range(6):
        try:
            res = run_bass_kernel_spmd(_NC_CACHE, in_maps,
                                       core_ids=list(range(NCORES)))
            break
        except Exception as e:  # transient NRT/terminal errors; retry w/ backoff
            last_err = e
            import time as _time
            _time.sleep(1.5 * (_attempt + 1))
            if _attempt >= 1:
                # same-executable retries can fail persistently; force a
                # fresh lowering + executable load (NEFF comes from the
                # on-disk compile cache, so this is cheap)
                try:
                    import jax as _jax
                    _jax.clear_caches()
                except Exception:
                    pass
                _NC_CACHE = _build()
    else:
        raise last_err

    y = np.empty((R, N, DIM), dtype=np.float32)
    for c in range(NCORES):
        ytc = res.results[c]["yt"].reshape(DIM, RL, N)  # row-group rg = r//8
        for rg in range(RL):
            y[c + 8 * rg] = ytc[:, rg, :].T
    return y
